# revision 23
# baseline (speedup 1.0000x reference)
"""DeepWalk loss kernel for 8 Trainium2 NeuronCores.

Strategy: data-parallel over the 512 walks (64 walks per core). Each core
compacts the referenced embedding rows into a DRAM token table (node||ctx,
512B per token) via windowed dma_gather (windows of 32768 rows so indices
fit int16; 1024-index single-packet calls through SBUF staging). All pair
operands are then fetched with 1024-index single-packet HBM dma_gather calls
in row layout (pairs on partitions), multiplied on DVE (bf16) and reduced
along the embedding axis into per-stream score strips; softplus is evaluated
with the Exp and Ln LUTs on the scalar engine with a fused row-sum
accumulator. Host sums the 8x[128] partial sums and divides by the pair
count.
"""

import os
import sys

import numpy as np
import ml_dtypes

sys.path.insert(0, "/opt/trn_rl_repo")

import concourse.bacc as bacc
import concourse.bass as bass
import concourse.mybir as mybir
import concourse.tile as tile
from concourse import library_config
from concourse.bass_utils import run_bass_kernel_spmd
from concourse._compat import with_exitstack

BF16 = ml_dtypes.bfloat16

# Problem constants (hardcoded per the harness contract).
EMB_DIM = 128
WALK_LEN = 40
WINDOW = 5
NEG_SIZE = 5
NUM_NODES = 1_000_000
BATCH = 512
N_CORES = 8

NB_CORE = BATCH // N_CORES            # 64 walks per core
NLOC = NB_CORE * WALK_LEN             # 2560 local walk positions
GTOK = BATCH * WALK_LEN               # 20480 global walk positions
P = 128
WIN_ROWS = 32768                      # int16-addressable window of the table
WCALL = 1024                          # idx per gather call (single-packet max)

def _pair_indices():
    src, dst = [], []
    for i in range(WALK_LEN):
        for j in range(max(0, i - WINDOW), i):
            src.append(j); dst.append(i)
        for j in range(i + 1, min(WALK_LEN, i + 1 + WINDOW)):
            src.append(j); dst.append(i)
    return np.asarray(src, dtype=np.int64), np.asarray(dst, dtype=np.int64)

_SRC, _DST = _pair_indices()
NUM_PAIRS = _SRC.shape[0]             # 370
POS_CORE = NB_CORE * NUM_PAIRS        # 23680 positive pairs per core
NPAD = 24064                          # pairs per stream, padded to 128 (188 cols)
NCOLS = NPAD // P                     # 188 score columns per stream
CALLS = [WCALL] * (NPAD // WCALL) + ([NPAD % WCALL] if NPAD % WCALL else [])
N_STREAMS = 6                         # pos, neg j=0..4
IDX_COLS = NPAD // 16                 # 1504 idx columns per list
N_LISTS = 7                           # AB, C, D0..D4

# --- experiment knobs (timing/ablation; defaults = production) ---
NO_COMPUTE = False      # skip mult/reduce/softplus (gathers only)
NO_MAIN = False         # skip the main pair gathers (build only)
QUEUES = 1              # SWDGE queues to round-robin gathers over
FAKE_WINDOWS = 0        # timing mode: N fake windows over a small vocab


def _wrap16(a):
    """int16 list [N] -> [128, N/16] dma_gather idx layout (16-wrap, 8x replicated)."""
    a = a.astype(np.int16)
    t = a.reshape(-1, 16).T          # [16, N/16]
    return np.tile(t, (8, 1)).copy() # [128, N/16]


def _plan(fw, vocab):
    """Window build plan: group the 20480 walk rows by table window.

    Each window is gathered with one WCALL(=1024)-index call into staging;
    the first padw(=896) slots are compacted into the DRAM table (token ids
    are padw*i + slot), the rest are pad reads of row 0.

    Returns (padw, bases, widx_lists, pos):
      bases[i]       HBM row base of window i's gather source slice
      widx_lists[i]  [WCALL] in-window row offsets (padded with 0)
      pos[w]         token id assigned to walk position w
    """
    if FAKE_WINDOWS:
        nwin = FAKE_WINDOWS
        win = np.argsort(np.argsort(fw, kind="stable"), kind="stable") % nwin
        bases = [0] * nwin
    else:
        nwin = (vocab + WIN_ROWS - 1) // WIN_ROWS
        win = fw // WIN_ROWS
        bases = [WIN_ROWS * i for i in range(nwin)]
    counts = np.bincount(win, minlength=nwin)
    padw = 896
    assert counts.max() <= min(padw, WCALL), f"window overflow: {counts.max()}"
    pos = np.empty(GTOK, dtype=np.int32)
    widx_lists = []
    for i in range(nwin):
        ws = np.nonzero(win == i)[0]
        pos[ws] = padw * i + np.arange(len(ws), dtype=np.int32)
        lst = np.zeros(WCALL, dtype=np.int32)
        lst[:len(ws)] = fw[ws] - bases[i]
        widx_lists.append(lst)
    return padw, bases, widx_lists, pos


def _host_prepare(batch_walk, neg_idx_dst, node_embed, context_embed):
    """Sharding/index prep. Index arithmetic + dtype casts only."""
    fw = np.asarray(batch_walk).reshape(-1).astype(np.int32)       # [20480]
    neg = np.asarray(neg_idx_dst).astype(np.int32)                 # [947200]
    vocab = int(np.asarray(node_embed).shape[0])

    tok = np.empty((vocab, 2 * EMB_DIM), dtype=BF16)
    tok[:, :EMB_DIM] = np.asarray(node_embed).astype(BF16)
    tok[:, EMB_DIM:] = np.asarray(context_embed).astype(BF16)

    padw, bases, widx_lists, pos = _plan(fw, vocab)
    nwin = len(bases)
    ntokb = padw * nwin
    pad_x, pad_y, pad_z = ntokb, ntokb + 1, ntokb + 2
    assert ntokb + 3 <= 32768, "token ids must fit int16"

    ptok = np.zeros((3, 2 * EMB_DIM), dtype=BF16)
    ptok[0, 0] = 1.0; ptok[0, EMB_DIM] = 30.0   # X: node=[1,..], ctx=[30,..]
    ptok[1, 0] = 1.0                            # Y: node=[1,..]
    ptok[2, EMB_DIM] = -30.0                    # Z: ctx=[-30,..]

    widx = np.concatenate([_wrap16(a) for a in widx_lists], axis=1)

    bl = np.repeat(np.arange(NB_CORE, dtype=np.int32), NUM_PAIRS)
    qq = np.tile(np.arange(NUM_PAIRS, dtype=np.int32), NB_CORE)
    npad = NPAD - POS_CORE

    in_maps = []
    for k in range(N_CORES):
        wloc = k * NLOC  # this core's batches start at walk position k*2560
        ab_t = pos[wloc + bl * WALK_LEN + _DST[qq].astype(np.int32)]
        c_t = pos[wloc + bl * WALK_LEN + _SRC[qq].astype(np.int32)]
        ab = np.concatenate([ab_t, np.full(npad, pad_x, np.int32)])
        cc = np.concatenate([c_t, np.full(npad, pad_y, np.int32)])
        negk = neg[k * POS_CORE * NEG_SIZE:(k + 1) * POS_CORE * NEG_SIZE]
        negk = negk.reshape(POS_CORE, NEG_SIZE)
        lists = [ab, cc]
        for j in range(NEG_SIZE):
            dj = np.concatenate([pos[negk[:, j]], np.full(npad, pad_z, np.int32)])
            lists.append(dj)
        gidx = np.concatenate([_wrap16(a) for a in lists], axis=1)  # [128, 7*1504]
        in_maps.append({"tok": tok, "widx": widx, "gidx": gidx, "ptok": ptok})
    return in_maps, padw, tuple(bases)


@with_exitstack
def _body(ctx, tc, nc, tok_t, widx_t, gidx_t, ptok_t, out_t, vocab, padw, bases):
    dt = mybir.dt
    nwin = len(bases)
    ntokb = padw * nwin
    wranks = padw // P                # 7 table ranks kept per window
    sranks = WCALL // P               # 8 staging ranks gathered per window
    wcols = WCALL // 16

    cst = ctx.enter_context(tc.tile_pool(name="cst", bufs=1))
    stg = ctx.enter_context(tc.tile_pool(name="stg", bufs=4))
    drm = ctx.enter_context(tc.tile_pool(name="drm", bufs=1, space="DRAM"))
    abp = ctx.enter_context(tc.tile_pool(name="apool", bufs=3))
    bp = ctx.enter_context(tc.tile_pool(name="bpool", bufs=3))
    cp = ctx.enter_context(tc.tile_pool(name="cpool", bufs=3))
    dp = ctx.enter_context(tc.tile_pool(name="dpool", bufs=4))
    scp = ctx.enter_context(tc.tile_pool(name="scr", bufs=2))

    dtab = drm.tile([ntokb + 3, 2 * EMB_DIM], dt.bfloat16)
    widx = cst.tile([P, nwin * wcols], dt.int16)
    gidx = cst.tile([P, N_LISTS * IDX_COLS], dt.int16)
    strips = []
    for si_ in range(N_STREAMS):
        strip = cst.tile([P, NCOLS], dt.float32, tag=f"strip{si_}", name=f"strip{si_}")
        strips.append(strip)

    nc.sync.dma_start(widx[:], widx_t[:])
    nc.sync.dma_start(gidx[:], gidx_t[:])

    qrr = [0]

    def nextq():
        return qrr[0] % QUEUES

    # Token-table build. Window i: one 1024-idx single-packet gather from its
    # HBM slice into a rotating staging tile (parallel across queues), then a
    # bulk DMA of the first 896 slots into the DRAM table. Pad tokens last.
    nc.sync.dma_start(
        dtab[ntokb:ntokb + 3, :].rearrange("(r p) e -> p r e", p=3), ptok_t[:])
    for i in range(nwin):
        lo = bases[i]
        hi = min(vocab, lo + WIN_ROWS)
        qrr[0] = i
        s = stg.tile([P, sranks, 2 * EMB_DIM], dt.bfloat16, tag="stg")
        nc.gpsimd.dma_gather(
            s[:], tok_t[lo:hi, :],
            widx[:, i * wcols:(i + 1) * wcols],
            WCALL, WCALL, 2 * EMB_DIM,
            single_packet=True,
            queue_num=nextq(),
        )
        nc.sync.dma_start(
            dtab[padw * i:padw * (i + 1), :].rearrange("(r p) e -> p r e", p=P),
            s[:, :wranks, :])

    node_half = dtab[:, :EMB_DIM]
    ctx_half = dtab[:, EMB_DIM:]

    def gather(dst, half, cols, n):
        nc.gpsimd.dma_gather(
            dst[:], half, gidx[:, cols:cols + n // 16], n, n, EMB_DIM,
            elem_step=2 * EMB_DIM,
            single_packet=True,
            queue_num=nextq(),
        )

    col0 = 0
    scol = 0
    for gi, n in enumerate(CALLS if not NO_MAIN else []):
        qrr[0] = gi  # all gathers of one call-group share a queue
        nr = n // P
        a = abp.tile([P, nr, EMB_DIM], dt.bfloat16, tag="a")
        gather(a, node_half, col0, n)
        b = bp.tile([P, nr, EMB_DIM], dt.bfloat16, tag="b")
        gather(b, ctx_half, col0, n)
        c = cp.tile([P, nr, EMB_DIM], dt.bfloat16, tag="c")
        gather(c, node_half, IDX_COLS + col0, n)
        ds = []
        for j in range(NEG_SIZE):
            d = dp.tile([P, nr, EMB_DIM], dt.bfloat16, tag="d")
            gather(d, ctx_half, (2 + j) * IDX_COLS + col0, n)
            ds.append(d)
        if not NO_COMPUTE:
            nc.vector.tensor_mul(c[:], c[:], b[:])
            nc.vector.tensor_reduce(strips[0][:, scol:scol + nr], c[:],
                                    axis=mybir.AxisListType.X,
                                    op=mybir.AluOpType.add)
            for j in range(NEG_SIZE):
                d = ds[j]
                nc.vector.tensor_mul(d[:], d[:], a[:])
                nc.vector.tensor_reduce(strips[1 + j][:, scol:scol + nr], d[:],
                                        axis=mybir.AxisListType.X,
                                        op=mybir.AluOpType.add)
        col0 += n // 16
        scol += nr

    accvs = []
    for s2 in range(N_STREAMS if not NO_COMPUTE and not NO_MAIN else 0):
        scale = -1.0 if s2 == 0 else 1.0   # pos stream: softplus(-score)
        e = scp.tile([P, NCOLS], dt.float32, tag="e")
        sp = scp.tile([P, NCOLS], dt.float32, tag="sp")
        av = cst.tile([P, 1], dt.float32, tag=f"av{s2}")
        nc.scalar.activation(e[:], strips[s2][:],
                             mybir.ActivationFunctionType.Exp, scale=scale)
        nc.scalar.activation(sp[:], e[:],
                             mybir.ActivationFunctionType.Ln,
                             bias=1.0, accum_out=av[:])
        accvs.append(av)
    osb = cst.tile([P, 1], dt.float32, tag="osb")
    nc.vector.memset(osb[:], 0.0)
    for av in accvs:
        nc.vector.tensor_add(osb[:], osb[:], av[:])
    nc.sync.dma_start(out_t[:], osb[:])


def _build_program(loop_k, vocab, padw, bases):
    nc = bacc.Bacc("TRN2", target_bir_lowering=False, debug=False,
                   num_swdge_queues=QUEUES)
    nwin = len(bases)
    tok_t = nc.dram_tensor("tok", [vocab, 2 * EMB_DIM], mybir.dt.bfloat16,
                           kind="ExternalInput")
    widx_t = nc.dram_tensor("widx", [P, nwin * WCALL // 16], mybir.dt.int16,
                            kind="ExternalInput")
    gidx_t = nc.dram_tensor("gidx", [P, N_LISTS * IDX_COLS], mybir.dt.int16,
                            kind="ExternalInput")
    ptok_t = nc.dram_tensor("ptok", [3, 2 * EMB_DIM], mybir.dt.bfloat16,
                            kind="ExternalInput")
    out_t = nc.dram_tensor("out", [P, 1], mybir.dt.float32, kind="ExternalOutput")
    with tile.TileContext(nc) as tc:
        nc.gpsimd.load_library(library_config.mlp)
        if loop_k is None:
            _body(tc, nc, tok_t, widx_t, gidx_t, ptok_t, out_t, vocab, padw, bases)
        else:
            tc.For_i_unrolled(0, loop_k, 1,
                              lambda iv: _body(tc, nc, tok_t, widx_t, gidx_t,
                                               ptok_t, out_t, vocab, padw, bases),
                              max_unroll=1)
    nc.compile()
    return nc


_CACHE = {}


def _get_program(loop_k, vocab, padw, bases):
    key = (loop_k, vocab, padw, bases, NO_COMPUTE, NO_MAIN, QUEUES)
    if key not in _CACHE:
        _CACHE[key] = _build_program(loop_k, vocab, padw, bases)
    return _CACHE[key]


def run_cores(inputs, loop_k=None):
    """Run the SPMD kernel; returns list of per-core [128,1] partial sums."""
    in_maps, padw, bases = _host_prepare(**inputs)
    vocab = int(np.asarray(inputs["node_embed"]).shape[0])
    nc = _get_program(loop_k, vocab, padw, bases)
    res = run_bass_kernel_spmd(nc, in_maps, core_ids=list(range(N_CORES)))
    return [res.results[i]["out"] for i in range(N_CORES)]


def kernel(batch_walk, neg_idx_dst, node_embed, context_embed):
    outs = run_cores(dict(batch_walk=batch_walk, neg_idx_dst=neg_idx_dst,
                          node_embed=node_embed, context_embed=context_embed))
    total = float(sum(float(o.sum()) for o in outs))
    return np.float32(total / (BATCH * NUM_PAIRS))


# revision 24
# speedup vs baseline: 2.5939x; 2.5939x over previous
"""DeepWalk loss kernel for 8 Trainium2 NeuronCores.

Strategy: data-parallel over the 512 walks (64 walks per core). Each core
compacts the referenced embedding rows into a DRAM token table (node||ctx,
512B per token) via windowed dma_gather (windows of 32768 rows so indices
fit int16; 1024-index single-packet calls through SBUF staging). All pair
operands are then fetched with 1024-index single-packet HBM dma_gather calls
in row layout (pairs on partitions), multiplied on DVE (bf16) and reduced
along the embedding axis into per-stream score strips; softplus is evaluated
with the Exp and Ln LUTs on the scalar engine with a fused row-sum
accumulator. Host sums the 8x[128] partial sums and divides by the pair
count.
"""

import os
import sys

import numpy as np
import ml_dtypes

sys.path.insert(0, "/opt/trn_rl_repo")

import concourse.bacc as bacc
import concourse.bass as bass
import concourse.mybir as mybir
import concourse.tile as tile
from concourse import library_config
from concourse.bass_utils import run_bass_kernel_spmd
from concourse._compat import with_exitstack
from concourse.tile import add_dep_helper

BF16 = ml_dtypes.bfloat16

# Problem constants (hardcoded per the harness contract).
EMB_DIM = 128
WALK_LEN = 40
WINDOW = 5
NEG_SIZE = 5
NUM_NODES = 1_000_000
BATCH = 512
N_CORES = 8

NB_CORE = BATCH // N_CORES            # 64 walks per core
NLOC = NB_CORE * WALK_LEN             # 2560 local walk positions
GTOK = BATCH * WALK_LEN               # 20480 global walk positions
P = 128
WIN_ROWS = 32768                      # int16-addressable window of the table
WCALL = 1024                          # idx per gather call (single-packet max)

def _pair_indices():
    src, dst = [], []
    for i in range(WALK_LEN):
        for j in range(max(0, i - WINDOW), i):
            src.append(j); dst.append(i)
        for j in range(i + 1, min(WALK_LEN, i + 1 + WINDOW)):
            src.append(j); dst.append(i)
    return np.asarray(src, dtype=np.int64), np.asarray(dst, dtype=np.int64)

_SRC, _DST = _pair_indices()
NUM_PAIRS = _SRC.shape[0]             # 370
POS_CORE = NB_CORE * NUM_PAIRS        # 23680 positive pairs per core
NPAD = 24064                          # pairs per stream, padded to 128 (188 cols)
NCOLS = NPAD // P                     # 188 score columns per stream
CALLS = [WCALL] * (NPAD // WCALL) + ([NPAD % WCALL] if NPAD % WCALL else [])
N_STREAMS = 6                         # pos, neg j=0..4
IDX_COLS = NPAD // 16                 # 1504 idx columns per list
N_LISTS = 7                           # AB, C, D0..D4

# --- experiment knobs (timing/ablation; defaults = production) ---
NO_COMPUTE = False      # skip mult/reduce/softplus (gathers only)
NO_MAIN = False         # skip the main pair gathers (build only)
QUEUES = 4              # SWDGE queues to round-robin gathers over
FAKE_WINDOWS = 0        # timing mode: N fake windows over a small vocab


def _wrap16(a):
    """int16 list [N] -> [128, N/16] dma_gather idx layout (16-wrap, 8x replicated)."""
    a = a.astype(np.int16)
    t = a.reshape(-1, 16).T          # [16, N/16]
    return np.tile(t, (8, 1)).copy() # [128, N/16]


def _plan(fw, vocab):
    """Window build plan: group the 20480 walk rows by table window.

    Each window is gathered with one WCALL(=1024)-index call into staging;
    the first padw(=896) slots are compacted into the DRAM table (token ids
    are padw*i + slot), the rest are pad reads of row 0.

    Returns (padw, bases, widx_lists, pos):
      bases[i]       HBM row base of window i's gather source slice
      widx_lists[i]  [WCALL] in-window row offsets (padded with 0)
      pos[w]         token id assigned to walk position w
    """
    if FAKE_WINDOWS:
        nwin = FAKE_WINDOWS
        win = np.argsort(np.argsort(fw, kind="stable"), kind="stable") % nwin
        bases = [0] * nwin
    else:
        nwin = (vocab + WIN_ROWS - 1) // WIN_ROWS
        win = fw // WIN_ROWS
        bases = [WIN_ROWS * i for i in range(nwin)]
    counts = np.bincount(win, minlength=nwin)
    padw = 896
    assert counts.max() <= min(padw, WCALL), f"window overflow: {counts.max()}"
    pos = np.empty(GTOK, dtype=np.int32)
    widx_lists = []
    for i in range(nwin):
        ws = np.nonzero(win == i)[0]
        pos[ws] = padw * i + np.arange(len(ws), dtype=np.int32)
        lst = np.zeros(WCALL, dtype=np.int32)
        lst[:len(ws)] = fw[ws] - bases[i]
        widx_lists.append(lst)
    return padw, bases, widx_lists, pos


def _host_prepare(batch_walk, neg_idx_dst, node_embed, context_embed):
    """Sharding/index prep. Index arithmetic + dtype casts only."""
    fw = np.asarray(batch_walk).reshape(-1).astype(np.int32)       # [20480]
    neg = np.asarray(neg_idx_dst).astype(np.int32)                 # [947200]
    vocab = int(np.asarray(node_embed).shape[0])

    tok = np.empty((vocab, 2 * EMB_DIM), dtype=BF16)
    tok[:, :EMB_DIM] = np.asarray(node_embed).astype(BF16)
    tok[:, EMB_DIM:] = np.asarray(context_embed).astype(BF16)

    padw, bases, widx_lists, pos = _plan(fw, vocab)
    nwin = len(bases)
    ntokb = padw * nwin
    pad_x, pad_y, pad_z = ntokb, ntokb + 1, ntokb + 2
    assert ntokb + 3 <= 32768, "token ids must fit int16"

    ptok = np.zeros((3, 2 * EMB_DIM), dtype=BF16)
    ptok[0, 0] = 1.0; ptok[0, EMB_DIM] = 30.0   # X: node=[1,..], ctx=[30,..]
    ptok[1, 0] = 1.0                            # Y: node=[1,..]
    ptok[2, EMB_DIM] = -30.0                    # Z: ctx=[-30,..]

    widx = np.concatenate([_wrap16(a) for a in widx_lists], axis=1)

    bl = np.repeat(np.arange(NB_CORE, dtype=np.int32), NUM_PAIRS)
    qq = np.tile(np.arange(NUM_PAIRS, dtype=np.int32), NB_CORE)
    npad = NPAD - POS_CORE

    in_maps = []
    for k in range(N_CORES):
        wloc = k * NLOC  # this core's batches start at walk position k*2560
        ab_t = pos[wloc + bl * WALK_LEN + _DST[qq].astype(np.int32)]
        c_t = pos[wloc + bl * WALK_LEN + _SRC[qq].astype(np.int32)]
        ab = np.concatenate([ab_t, np.full(npad, pad_x, np.int32)])
        cc = np.concatenate([c_t, np.full(npad, pad_y, np.int32)])
        negk = neg[k * POS_CORE * NEG_SIZE:(k + 1) * POS_CORE * NEG_SIZE]
        negk = negk.reshape(POS_CORE, NEG_SIZE)
        lists = [ab, cc]
        for j in range(NEG_SIZE):
            dj = np.concatenate([pos[negk[:, j]], np.full(npad, pad_z, np.int32)])
            lists.append(dj)
        gidx = np.concatenate([_wrap16(a) for a in lists], axis=1)  # [128, 7*1504]
        in_maps.append({"tok": tok, "widx": widx, "gidx": gidx, "ptok": ptok})
    return in_maps, padw, tuple(bases)


@with_exitstack
def _body(ctx, tc, nc, tok_t, widx_t, gidx_t, ptok_t, out_t, vocab, padw, bases):
    dt = mybir.dt
    nwin = len(bases)
    ntokb = padw * nwin
    wranks = padw // P                # 7 table ranks kept per window
    sranks = WCALL // P               # 8 staging ranks gathered per window
    wcols = WCALL // 16

    cst = ctx.enter_context(tc.tile_pool(name="cst", bufs=1))
    stg = ctx.enter_context(tc.tile_pool(name="stg", bufs=4))
    drm = ctx.enter_context(tc.tile_pool(name="drm", bufs=1, space="DRAM"))
    abp = ctx.enter_context(tc.tile_pool(name="apool", bufs=3))
    bp = ctx.enter_context(tc.tile_pool(name="bpool", bufs=3))
    cp = ctx.enter_context(tc.tile_pool(name="cpool", bufs=3))
    dp = ctx.enter_context(tc.tile_pool(name="dpool", bufs=4))
    scp = ctx.enter_context(tc.tile_pool(name="scr", bufs=2))

    dtab = drm.tile([ntokb + 3, 2 * EMB_DIM], dt.bfloat16)
    widx = cst.tile([P, nwin * wcols], dt.int16)
    gidx = cst.tile([P, N_LISTS * IDX_COLS], dt.int16)
    strips = []
    for si_ in range(N_STREAMS):
        strip = cst.tile([P, NCOLS], dt.float32, tag=f"strip{si_}", name=f"strip{si_}")
        strips.append(strip)

    nc.sync.dma_start(widx[:], widx_t[:])
    nc.sync.dma_start(gidx[:], gidx_t[:])

    qrr = [0]
    prev_g = [None]

    def nextq():
        q = qrr[0] % QUEUES
        qrr[0] += 1
        return q

    def chain(inst):
        # Pin scheduler emission order of SWDGE ops to program order so
        # Tile's DMA sem lanes (rr mod 8) stay aligned with the strict
        # queue round-robin (mod 4): ucode requires each sem lane to be
        # incremented from a single queue.
        if prev_g[0] is not None:
            add_dep_helper(inst.ins, prev_g[0].ins, False,
                           "swdge order chain")
        prev_g[0] = inst

    # Token-table build. Window i: one 1024-idx single-packet gather from its
    # HBM slice into a rotating staging tile (parallel across queues), then a
    # bulk DMA of the first 896 slots into the DRAM table. Pad tokens last.
    nc.sync.dma_start(
        dtab[ntokb:ntokb + 3, :].rearrange("(r p) e -> p r e", p=3), ptok_t[:])
    for i in range(nwin):
        lo = bases[i]
        hi = min(vocab, lo + WIN_ROWS)
        s = stg.tile([P, sranks, 2 * EMB_DIM], dt.bfloat16, tag="stg")
        g = nc.gpsimd.dma_gather(
            s[:], tok_t[lo:hi, :],
            widx[:, i * wcols:(i + 1) * wcols],
            WCALL, WCALL, 2 * EMB_DIM,
            single_packet=True,
            queue_num=nextq(),
        )
        chain(g)
        nc.sync.dma_start(
            dtab[padw * i:padw * (i + 1), :].rearrange("(r p) e -> p r e", p=P),
            s[:, :wranks, :])

    node_half = dtab[:, :EMB_DIM]
    ctx_half = dtab[:, EMB_DIM:]

    def gather(dst, half, cols, n):
        g = nc.gpsimd.dma_gather(
            dst[:], half, gidx[:, cols:cols + n // 16], n, n, EMB_DIM,
            elem_step=2 * EMB_DIM,
            single_packet=True,
            queue_num=nextq(),
        )
        chain(g)

    col0 = 0
    scol = 0
    for gi, n in enumerate(CALLS if not NO_MAIN else []):
        nr = n // P
        a = abp.tile([P, nr, EMB_DIM], dt.bfloat16, tag="a")
        gather(a, node_half, col0, n)
        b = bp.tile([P, nr, EMB_DIM], dt.bfloat16, tag="b")
        gather(b, ctx_half, col0, n)
        c = cp.tile([P, nr, EMB_DIM], dt.bfloat16, tag="c")
        gather(c, node_half, IDX_COLS + col0, n)
        ds = []
        for j in range(NEG_SIZE):
            d = dp.tile([P, nr, EMB_DIM], dt.bfloat16, tag="d")
            gather(d, ctx_half, (2 + j) * IDX_COLS + col0, n)
            ds.append(d)
        if not NO_COMPUTE:
            nc.vector.tensor_mul(c[:], c[:], b[:])
            nc.vector.tensor_reduce(strips[0][:, scol:scol + nr], c[:],
                                    axis=mybir.AxisListType.X,
                                    op=mybir.AluOpType.add)
            for j in range(NEG_SIZE):
                d = ds[j]
                nc.vector.tensor_mul(d[:], d[:], a[:])
                nc.vector.tensor_reduce(strips[1 + j][:, scol:scol + nr], d[:],
                                        axis=mybir.AxisListType.X,
                                        op=mybir.AluOpType.add)
        col0 += n // 16
        scol += nr

    accvs = []
    for s2 in range(N_STREAMS if not NO_COMPUTE and not NO_MAIN else 0):
        scale = -1.0 if s2 == 0 else 1.0   # pos stream: softplus(-score)
        e = scp.tile([P, NCOLS], dt.float32, tag="e")
        sp = scp.tile([P, NCOLS], dt.float32, tag="sp")
        av = cst.tile([P, 1], dt.float32, tag=f"av{s2}")
        nc.scalar.activation(e[:], strips[s2][:],
                             mybir.ActivationFunctionType.Exp, scale=scale)
        nc.scalar.activation(sp[:], e[:],
                             mybir.ActivationFunctionType.Ln,
                             bias=1.0, accum_out=av[:])
        accvs.append(av)
    osb = cst.tile([P, 1], dt.float32, tag="osb")
    nc.vector.memset(osb[:], 0.0)
    for av in accvs:
        nc.vector.tensor_add(osb[:], osb[:], av[:])
    nc.sync.dma_start(out_t[:], osb[:])


def _build_program(loop_k, vocab, padw, bases):
    nc = bacc.Bacc("TRN2", target_bir_lowering=False, debug=False,
                   num_swdge_queues=QUEUES)
    nwin = len(bases)
    tok_t = nc.dram_tensor("tok", [vocab, 2 * EMB_DIM], mybir.dt.bfloat16,
                           kind="ExternalInput")
    widx_t = nc.dram_tensor("widx", [P, nwin * WCALL // 16], mybir.dt.int16,
                            kind="ExternalInput")
    gidx_t = nc.dram_tensor("gidx", [P, N_LISTS * IDX_COLS], mybir.dt.int16,
                            kind="ExternalInput")
    ptok_t = nc.dram_tensor("ptok", [3, 2 * EMB_DIM], mybir.dt.bfloat16,
                            kind="ExternalInput")
    out_t = nc.dram_tensor("out", [P, 1], mybir.dt.float32, kind="ExternalOutput")
    with tile.TileContext(nc) as tc:
        nc.gpsimd.load_library(library_config.mlp)
        if loop_k is None:
            _body(tc, nc, tok_t, widx_t, gidx_t, ptok_t, out_t, vocab, padw, bases)
        else:
            tc.For_i_unrolled(0, loop_k, 1,
                              lambda iv: _body(tc, nc, tok_t, widx_t, gidx_t,
                                               ptok_t, out_t, vocab, padw, bases),
                              max_unroll=1)
    nc.compile()
    return nc


_CACHE = {}


def _get_program(loop_k, vocab, padw, bases):
    key = (loop_k, vocab, padw, bases, NO_COMPUTE, NO_MAIN, QUEUES)
    if key not in _CACHE:
        _CACHE[key] = _build_program(loop_k, vocab, padw, bases)
    return _CACHE[key]


def run_cores(inputs, loop_k=None):
    """Run the SPMD kernel; returns list of per-core [128,1] partial sums."""
    in_maps, padw, bases = _host_prepare(**inputs)
    vocab = int(np.asarray(inputs["node_embed"]).shape[0])
    nc = _get_program(loop_k, vocab, padw, bases)
    res = run_bass_kernel_spmd(nc, in_maps, core_ids=list(range(N_CORES)))
    return [res.results[i]["out"] for i in range(N_CORES)]


def kernel(batch_walk, neg_idx_dst, node_embed, context_embed):
    outs = run_cores(dict(batch_walk=batch_walk, neg_idx_dst=neg_idx_dst,
                          node_embed=node_embed, context_embed=context_embed))
    total = float(sum(float(o.sum()) for o in outs))
    return np.float32(total / (BATCH * NUM_PAIRS))


# revision 25
# speedup vs baseline: 2.7426x; 1.0573x over previous
"""DeepWalk loss kernel for 8 Trainium2 NeuronCores.

Strategy: data-parallel over the 512 walks (64 walks per core). Each core
compacts the referenced embedding rows into a DRAM token table (node||ctx,
512B per token) via windowed dma_gather (windows of 32768 rows so indices
fit int16; 1024-index single-packet calls through SBUF staging). All pair
operands are then fetched with 1024-index single-packet HBM dma_gather calls
in row layout (pairs on partitions), multiplied on DVE (bf16) and reduced
along the embedding axis into per-stream score strips; softplus is evaluated
with the Exp and Ln LUTs on the scalar engine with a fused row-sum
accumulator. Host sums the 8x[128] partial sums and divides by the pair
count.
"""

import os
import sys

import numpy as np
import ml_dtypes

sys.path.insert(0, "/opt/trn_rl_repo")

import concourse.bacc as bacc
import concourse.bass as bass
import concourse.mybir as mybir
import concourse.tile as tile
from concourse import library_config
from concourse.bass_utils import run_bass_kernel_spmd
from concourse._compat import with_exitstack
from concourse.tile import add_dep_helper

BF16 = ml_dtypes.bfloat16

# Problem constants (hardcoded per the harness contract).
EMB_DIM = 128
WALK_LEN = 40
WINDOW = 5
NEG_SIZE = 5
NUM_NODES = 1_000_000
BATCH = 512
N_CORES = 8

NB_CORE = BATCH // N_CORES            # 64 walks per core
NLOC = NB_CORE * WALK_LEN             # 2560 local walk positions
GTOK = BATCH * WALK_LEN               # 20480 global walk positions
P = 128
WIN_ROWS = 32768                      # int16-addressable window of the table
WCALL = 1024                          # idx per gather call (single-packet max)

def _pair_indices():
    src, dst = [], []
    for i in range(WALK_LEN):
        for j in range(max(0, i - WINDOW), i):
            src.append(j); dst.append(i)
        for j in range(i + 1, min(WALK_LEN, i + 1 + WINDOW)):
            src.append(j); dst.append(i)
    return np.asarray(src, dtype=np.int64), np.asarray(dst, dtype=np.int64)

_SRC, _DST = _pair_indices()
NUM_PAIRS = _SRC.shape[0]             # 370
POS_CORE = NB_CORE * NUM_PAIRS        # 23680 positive pairs per core
NPAD = 24064                          # pairs per stream, padded to 128 (188 cols)
NCOLS = NPAD // P                     # 188 score columns per stream
CALLS = [WCALL] * (NPAD // WCALL) + ([NPAD % WCALL] if NPAD % WCALL else [])
N_STREAMS = 6                         # pos, neg j=0..4
IDX_COLS = NPAD // 16                 # 1504 idx columns per list
N_LISTS = 7                           # AB, C, D0..D4

# --- experiment knobs (timing/ablation; defaults = production) ---
NO_COMPUTE = False      # skip mult/reduce/softplus (gathers only)
NO_MAIN = False         # skip the main pair gathers (build only)
QUEUES = 4              # SWDGE queues to round-robin gathers over
FAKE_WINDOWS = 0        # timing mode: N fake windows over a small vocab


def _wrap16(a):
    """int16 list [N] -> [128, N/16] dma_gather idx layout (16-wrap, 8x replicated)."""
    a = a.astype(np.int16)
    t = a.reshape(-1, 16).T          # [16, N/16]
    return np.tile(t, (8, 1)).copy() # [128, N/16]


def _plan(fw, vocab):
    """Window build plan: group the 20480 walk rows by table window.

    Each window is gathered with one WCALL(=1024)-index call into staging;
    the first padw(=896) slots are compacted into the DRAM table (token ids
    are padw*i + slot), the rest are pad reads of row 0.

    Returns (padw, bases, widx_lists, pos):
      bases[i]       HBM row base of window i's gather source slice
      widx_lists[i]  [WCALL] in-window row offsets (padded with 0)
      pos[w]         token id assigned to walk position w
    """
    if FAKE_WINDOWS:
        nwin = FAKE_WINDOWS
        win = np.argsort(np.argsort(fw, kind="stable"), kind="stable") % nwin
        bases = [0] * nwin
    else:
        nwin = (vocab + WIN_ROWS - 1) // WIN_ROWS
        win = fw // WIN_ROWS
        bases = [WIN_ROWS * i for i in range(nwin)]
    counts = np.bincount(win, minlength=nwin)
    padw = 896
    assert counts.max() <= min(padw, WCALL), f"window overflow: {counts.max()}"
    pos = np.empty(GTOK, dtype=np.int32)
    widx_lists = []
    for i in range(nwin):
        ws = np.nonzero(win == i)[0]
        pos[ws] = padw * i + np.arange(len(ws), dtype=np.int32)
        lst = np.zeros(WCALL, dtype=np.int32)
        lst[:len(ws)] = fw[ws] - bases[i]
        widx_lists.append(lst)
    return padw, bases, widx_lists, pos


def _host_prepare(batch_walk, neg_idx_dst, node_embed, context_embed):
    """Sharding/index prep. Index arithmetic + dtype casts only."""
    fw = np.asarray(batch_walk).reshape(-1).astype(np.int32)       # [20480]
    neg = np.asarray(neg_idx_dst).astype(np.int32)                 # [947200]
    vocab = int(np.asarray(node_embed).shape[0])

    tok = np.empty((vocab, 2 * EMB_DIM), dtype=BF16)
    tok[:, :EMB_DIM] = np.asarray(node_embed).astype(BF16)
    tok[:, EMB_DIM:] = np.asarray(context_embed).astype(BF16)

    padw, bases, widx_lists, pos = _plan(fw, vocab)
    nwin = len(bases)
    ntokb = padw * nwin
    pad_x, pad_y, pad_z = ntokb, ntokb + 1, ntokb + 2
    assert ntokb + 3 <= 32768, "token ids must fit int16"

    ptok = np.zeros((3, 2 * EMB_DIM), dtype=BF16)
    ptok[0, 0] = 1.0; ptok[0, EMB_DIM] = 30.0   # X: node=[1,..], ctx=[30,..]
    ptok[1, 0] = 1.0                            # Y: node=[1,..]
    ptok[2, EMB_DIM] = -30.0                    # Z: ctx=[-30,..]

    widx = np.concatenate([_wrap16(a) for a in widx_lists], axis=1)

    bl = np.repeat(np.arange(NB_CORE, dtype=np.int32), NUM_PAIRS)
    qq = np.tile(np.arange(NUM_PAIRS, dtype=np.int32), NB_CORE)
    npad = NPAD - POS_CORE

    in_maps = []
    for k in range(N_CORES):
        wloc = k * NLOC  # this core's batches start at walk position k*2560
        ab_t = pos[wloc + bl * WALK_LEN + _DST[qq].astype(np.int32)]
        c_t = pos[wloc + bl * WALK_LEN + _SRC[qq].astype(np.int32)]
        ab = np.concatenate([ab_t, np.full(npad, pad_x, np.int32)])
        cc = np.concatenate([c_t, np.full(npad, pad_y, np.int32)])
        negk = neg[k * POS_CORE * NEG_SIZE:(k + 1) * POS_CORE * NEG_SIZE]
        negk = negk.reshape(POS_CORE, NEG_SIZE)
        lists = [ab, cc]
        for j in range(NEG_SIZE):
            dj = np.concatenate([pos[negk[:, j]], np.full(npad, pad_z, np.int32)])
            lists.append(dj)
        gidx = np.concatenate([_wrap16(a) for a in lists], axis=1)  # [128, 7*1504]
        in_maps.append({"tok": tok, "widx": widx, "gidx": gidx, "ptok": ptok})
    return in_maps, padw, tuple(bases)


@with_exitstack
def _body(ctx, tc, nc, tok_t, widx_t, gidx_t, ptok_t, out_t, vocab, padw, bases):
    dt = mybir.dt
    nwin = len(bases)
    ntokb = padw * nwin
    wranks = padw // P                # 7 table ranks kept per window
    sranks = WCALL // P               # 8 staging ranks gathered per window
    wcols = WCALL // 16

    cst = ctx.enter_context(tc.tile_pool(name="cst", bufs=1))
    stg = ctx.enter_context(tc.tile_pool(name="stg", bufs=8))
    drm = ctx.enter_context(tc.tile_pool(name="drm", bufs=1, space="DRAM"))
    abp = ctx.enter_context(tc.tile_pool(name="apool", bufs=6))
    bp = ctx.enter_context(tc.tile_pool(name="bpool", bufs=6))
    cp = ctx.enter_context(tc.tile_pool(name="cpool", bufs=6))
    dp = ctx.enter_context(tc.tile_pool(name="dpool", bufs=15))
    scp = ctx.enter_context(tc.tile_pool(name="scr", bufs=3))

    dtab = drm.tile([ntokb + 3, 2 * EMB_DIM], dt.bfloat16)
    widx = cst.tile([P, nwin * wcols], dt.int16)
    gidx = cst.tile([P, N_LISTS * IDX_COLS], dt.int16)
    strips = []
    for si_ in range(N_STREAMS):
        strip = cst.tile([P, NCOLS], dt.float32, tag=f"strip{si_}", name=f"strip{si_}")
        strips.append(strip)

    nc.sync.dma_start(widx[:], widx_t[:])
    nc.sync.dma_start(gidx[:], gidx_t[:])

    qrr = [0]
    prev_g = [None]

    def nextq():
        q = qrr[0] % QUEUES
        qrr[0] += 1
        return q

    def chain(inst):
        # Pin scheduler emission order of SWDGE ops to program order so
        # Tile's DMA sem lanes (rr mod 8) stay aligned with the strict
        # queue round-robin (mod 4): ucode requires each sem lane to be
        # incremented from a single queue.
        if prev_g[0] is not None:
            add_dep_helper(inst.ins, prev_g[0].ins, False,
                           "swdge order chain")
        prev_g[0] = inst

    # Token-table build. Window i: one 1024-idx single-packet gather from its
    # HBM slice into a rotating staging tile (parallel across queues), then a
    # bulk DMA of the first 896 slots into the DRAM table. Pad tokens last.
    nc.sync.dma_start(
        dtab[ntokb:ntokb + 3, :].rearrange("(r p) e -> p r e", p=3), ptok_t[:])
    for i in range(nwin):
        lo = bases[i]
        hi = min(vocab, lo + WIN_ROWS)
        s = stg.tile([P, sranks, 2 * EMB_DIM], dt.bfloat16, tag="stg")
        g = nc.gpsimd.dma_gather(
            s[:], tok_t[lo:hi, :],
            widx[:, i * wcols:(i + 1) * wcols],
            WCALL, WCALL, 2 * EMB_DIM,
            single_packet=True,
            queue_num=nextq(),
        )
        chain(g)
        nc.sync.dma_start(
            dtab[padw * i:padw * (i + 1), :].rearrange("(r p) e -> p r e", p=P),
            s[:, :wranks, :])

    node_half = dtab[:, :EMB_DIM]
    ctx_half = dtab[:, EMB_DIM:]

    def gather(dst, half, cols, n):
        g = nc.gpsimd.dma_gather(
            dst[:], half, gidx[:, cols:cols + n // 16], n, n, EMB_DIM,
            elem_step=2 * EMB_DIM,
            single_packet=True,
            queue_num=nextq(),
        )
        chain(g)

    col0 = 0
    scol = 0
    for gi, n in enumerate(CALLS if not NO_MAIN else []):
        nr = n // P
        a = abp.tile([P, nr, EMB_DIM], dt.bfloat16, tag="a")
        gather(a, node_half, col0, n)
        b = bp.tile([P, nr, EMB_DIM], dt.bfloat16, tag="b")
        gather(b, ctx_half, col0, n)
        c = cp.tile([P, nr, EMB_DIM], dt.bfloat16, tag="c")
        gather(c, node_half, IDX_COLS + col0, n)
        ds = []
        for j in range(NEG_SIZE):
            d = dp.tile([P, nr, EMB_DIM], dt.bfloat16, tag="d")
            gather(d, ctx_half, (2 + j) * IDX_COLS + col0, n)
            ds.append(d)
        if not NO_COMPUTE:
            nc.vector.tensor_mul(c[:], c[:], b[:])
            nc.vector.tensor_reduce(strips[0][:, scol:scol + nr], c[:],
                                    axis=mybir.AxisListType.X,
                                    op=mybir.AluOpType.add)
            for j in range(NEG_SIZE):
                d = ds[j]
                nc.vector.tensor_mul(d[:], d[:], a[:])
                nc.vector.tensor_reduce(strips[1 + j][:, scol:scol + nr], d[:],
                                        axis=mybir.AxisListType.X,
                                        op=mybir.AluOpType.add)
        col0 += n // 16
        scol += nr

    accvs = []
    for s2 in range(N_STREAMS if not NO_COMPUTE and not NO_MAIN else 0):
        scale = -1.0 if s2 == 0 else 1.0   # pos stream: softplus(-score)
        e = scp.tile([P, NCOLS], dt.float32, tag="e")
        sp = scp.tile([P, NCOLS], dt.float32, tag="sp")
        av = cst.tile([P, 1], dt.float32, tag=f"av{s2}")
        nc.scalar.activation(e[:], strips[s2][:],
                             mybir.ActivationFunctionType.Exp, scale=scale)
        nc.scalar.activation(sp[:], e[:],
                             mybir.ActivationFunctionType.Ln,
                             bias=1.0, accum_out=av[:])
        accvs.append(av)
    osb = cst.tile([P, 1], dt.float32, tag="osb")
    nc.vector.memset(osb[:], 0.0)
    for av in accvs:
        nc.vector.tensor_add(osb[:], osb[:], av[:])
    nc.sync.dma_start(out_t[:], osb[:])


def _build_program(loop_k, vocab, padw, bases):
    nc = bacc.Bacc("TRN2", target_bir_lowering=False, debug=False,
                   num_swdge_queues=QUEUES)
    nwin = len(bases)
    tok_t = nc.dram_tensor("tok", [vocab, 2 * EMB_DIM], mybir.dt.bfloat16,
                           kind="ExternalInput")
    widx_t = nc.dram_tensor("widx", [P, nwin * WCALL // 16], mybir.dt.int16,
                            kind="ExternalInput")
    gidx_t = nc.dram_tensor("gidx", [P, N_LISTS * IDX_COLS], mybir.dt.int16,
                            kind="ExternalInput")
    ptok_t = nc.dram_tensor("ptok", [3, 2 * EMB_DIM], mybir.dt.bfloat16,
                            kind="ExternalInput")
    out_t = nc.dram_tensor("out", [P, 1], mybir.dt.float32, kind="ExternalOutput")
    with tile.TileContext(nc) as tc:
        nc.gpsimd.load_library(library_config.mlp)
        if loop_k is None:
            _body(tc, nc, tok_t, widx_t, gidx_t, ptok_t, out_t, vocab, padw, bases)
        else:
            tc.For_i_unrolled(0, loop_k, 1,
                              lambda iv: _body(tc, nc, tok_t, widx_t, gidx_t,
                                               ptok_t, out_t, vocab, padw, bases),
                              max_unroll=1)
    nc.compile()
    return nc


_CACHE = {}


def _get_program(loop_k, vocab, padw, bases):
    key = (loop_k, vocab, padw, bases, NO_COMPUTE, NO_MAIN, QUEUES)
    if key not in _CACHE:
        _CACHE[key] = _build_program(loop_k, vocab, padw, bases)
    return _CACHE[key]


def run_cores(inputs, loop_k=None):
    """Run the SPMD kernel; returns list of per-core [128,1] partial sums."""
    in_maps, padw, bases = _host_prepare(**inputs)
    vocab = int(np.asarray(inputs["node_embed"]).shape[0])
    nc = _get_program(loop_k, vocab, padw, bases)
    res = run_bass_kernel_spmd(nc, in_maps, core_ids=list(range(N_CORES)))
    return [res.results[i]["out"] for i in range(N_CORES)]


def kernel(batch_walk, neg_idx_dst, node_embed, context_embed):
    outs = run_cores(dict(batch_walk=batch_walk, neg_idx_dst=neg_idx_dst,
                          node_embed=node_embed, context_embed=context_embed))
    total = float(sum(float(o.sum()) for o in outs))
    return np.float32(total / (BATCH * NUM_PAIRS))


# revision 26
# speedup vs baseline: 3.0347x; 1.1065x over previous
"""DeepWalk loss kernel for 8 Trainium2 NeuronCores.

Strategy: data-parallel over the 512 walks (64 walks per core). Each core
compacts the referenced embedding rows into a DRAM token table (node||ctx,
512B per token) via windowed dma_gather (windows of 32768 rows so indices
fit int16; 1024-index single-packet calls through SBUF staging). All pair
operands are then fetched with 1024-index single-packet HBM dma_gather calls
in row layout (pairs on partitions), multiplied on DVE (bf16) and reduced
along the embedding axis into per-stream score strips; softplus is evaluated
with the Exp and Ln LUTs on the scalar engine with a fused row-sum
accumulator. Host sums the 8x[128] partial sums and divides by the pair
count.
"""

import os
import sys

import numpy as np
import ml_dtypes

sys.path.insert(0, "/opt/trn_rl_repo")

import concourse.bacc as bacc
import concourse.bass as bass
import concourse.mybir as mybir
import concourse.tile as tile
from concourse import library_config
from concourse.bass_utils import run_bass_kernel_spmd
from concourse._compat import with_exitstack
from concourse.tile import add_dep_helper

BF16 = ml_dtypes.bfloat16

# Problem constants (hardcoded per the harness contract).
EMB_DIM = 128
WALK_LEN = 40
WINDOW = 5
NEG_SIZE = 5
NUM_NODES = 1_000_000
BATCH = 512
N_CORES = 8

NB_CORE = BATCH // N_CORES            # 64 walks per core
NLOC = NB_CORE * WALK_LEN             # 2560 local walk positions
GTOK = BATCH * WALK_LEN               # 20480 global walk positions
P = 128
WIN_ROWS = 32768                      # int16-addressable window of the table
WCALL = 1024                          # idx per gather call (single-packet max)

def _pair_indices():
    src, dst = [], []
    for i in range(WALK_LEN):
        for j in range(max(0, i - WINDOW), i):
            src.append(j); dst.append(i)
        for j in range(i + 1, min(WALK_LEN, i + 1 + WINDOW)):
            src.append(j); dst.append(i)
    return np.asarray(src, dtype=np.int64), np.asarray(dst, dtype=np.int64)

_SRC, _DST = _pair_indices()
NUM_PAIRS = _SRC.shape[0]             # 370
POS_CORE = NB_CORE * NUM_PAIRS        # 23680 positive pairs per core
NPAD = 24064                          # pairs per stream, padded to 128 (188 cols)
NCOLS = NPAD // P                     # 188 score columns per stream
CALLS = [WCALL] * (NPAD // WCALL) + ([NPAD % WCALL] if NPAD % WCALL else [])
N_STREAMS = 6                         # pos, neg j=0..4
IDX_COLS = NPAD // 16                 # 1504 idx columns per list
N_LISTS = 7                           # AB, C, D0..D4

# --- experiment knobs (timing/ablation; defaults = production) ---
NO_COMPUTE = False      # skip mult/reduce/softplus (gathers only)
NO_MAIN = False         # skip the main pair gathers (build only)
QUEUES = 4              # SWDGE queues to round-robin gathers over
FAKE_WINDOWS = 0        # timing mode: N fake windows over a small vocab


def _wrap16(a):
    """int16 list [N] -> [128, N/16] dma_gather idx layout (16-wrap, 8x replicated)."""
    a = a.astype(np.int16)
    t = a.reshape(-1, 16).T          # [16, N/16]
    return np.tile(t, (8, 1)).copy() # [128, N/16]


def _plan(fw, vocab):
    """Window build plan: group the 20480 walk rows by table window.

    Each window is gathered with one WCALL(=1024)-index call into staging;
    the first padw(=896) slots are compacted into the DRAM table (token ids
    are padw*i + slot), the rest are pad reads of row 0.

    Returns (padw, bases, widx_lists, pos):
      bases[i]       HBM row base of window i's gather source slice
      widx_lists[i]  [WCALL] in-window row offsets (padded with 0)
      pos[w]         token id assigned to walk position w
    """
    if FAKE_WINDOWS:
        nwin = FAKE_WINDOWS
        win = np.argsort(np.argsort(fw, kind="stable"), kind="stable") % nwin
        bases = [0] * nwin
    else:
        nwin = (vocab + WIN_ROWS - 1) // WIN_ROWS
        win = fw // WIN_ROWS
        bases = [WIN_ROWS * i for i in range(nwin)]
    counts = np.bincount(win, minlength=nwin)
    padw = 896
    assert counts.max() <= min(padw, WCALL), f"window overflow: {counts.max()}"
    pos = np.empty(GTOK, dtype=np.int32)
    widx_lists = []
    for i in range(nwin):
        ws = np.nonzero(win == i)[0]
        pos[ws] = padw * i + np.arange(len(ws), dtype=np.int32)
        lst = np.zeros(WCALL, dtype=np.int32)
        lst[:len(ws)] = fw[ws] - bases[i]
        widx_lists.append(lst)
    return padw, bases, widx_lists, pos


def _host_prepare(batch_walk, neg_idx_dst, node_embed, context_embed):
    """Sharding/index prep. Index arithmetic + dtype casts only."""
    fw = np.asarray(batch_walk).reshape(-1).astype(np.int32)       # [20480]
    neg = np.asarray(neg_idx_dst).astype(np.int32)                 # [947200]
    vocab = int(np.asarray(node_embed).shape[0])

    tok = np.empty((vocab, 2 * EMB_DIM), dtype=BF16)
    tok[:, :EMB_DIM] = np.asarray(node_embed).astype(BF16)
    tok[:, EMB_DIM:] = np.asarray(context_embed).astype(BF16)

    padw, bases, widx_lists, pos = _plan(fw, vocab)
    nwin = len(bases)
    ntokb = padw * nwin
    pad_x, pad_y, pad_z = ntokb, ntokb + 1, ntokb + 2
    assert ntokb + 3 <= 32768, "token ids must fit int16"

    ptok = np.zeros((3, 2 * EMB_DIM), dtype=BF16)
    ptok[0, 0] = 1.0; ptok[0, EMB_DIM] = 30.0   # X: node=[1,..], ctx=[30,..]
    ptok[1, 0] = 1.0                            # Y: node=[1,..]
    ptok[2, EMB_DIM] = -30.0                    # Z: ctx=[-30,..]

    widx = np.concatenate([_wrap16(a) for a in widx_lists], axis=1)

    bl = np.repeat(np.arange(NB_CORE, dtype=np.int32), NUM_PAIRS)
    qq = np.tile(np.arange(NUM_PAIRS, dtype=np.int32), NB_CORE)
    npad = NPAD - POS_CORE

    in_maps = []
    for k in range(N_CORES):
        wloc = k * NLOC  # this core's batches start at walk position k*2560
        ab_t = pos[wloc + bl * WALK_LEN + _DST[qq].astype(np.int32)]
        c_t = pos[wloc + bl * WALK_LEN + _SRC[qq].astype(np.int32)]
        ab = np.concatenate([ab_t, np.full(npad, pad_x, np.int32)])
        cc = np.concatenate([c_t, np.full(npad, pad_y, np.int32)])
        negk = neg[k * POS_CORE * NEG_SIZE:(k + 1) * POS_CORE * NEG_SIZE]
        negk = negk.reshape(POS_CORE, NEG_SIZE)
        lists = [ab, cc]
        for j in range(NEG_SIZE):
            dj = np.concatenate([pos[negk[:, j]], np.full(npad, pad_z, np.int32)])
            lists.append(dj)
        gidx = np.concatenate([_wrap16(a) for a in lists], axis=1)  # [128, 7*1504]
        in_maps.append({"tok": tok, "widx": widx, "gidx": gidx, "ptok": ptok})
    return in_maps, padw, tuple(bases)


@with_exitstack
def _body(ctx, tc, nc, tok_t, widx_t, gidx_t, ptok_t, out_t, vocab, padw, bases):
    dt = mybir.dt
    nwin = len(bases)
    ntokb = padw * nwin
    wranks = padw // P                # 7 table ranks kept per window
    sranks = WCALL // P               # 8 staging ranks gathered per window
    wcols = WCALL // 16

    cst = ctx.enter_context(tc.tile_pool(name="cst", bufs=1))
    stg = ctx.enter_context(tc.tile_pool(name="stg", bufs=12))
    drm = ctx.enter_context(tc.tile_pool(name="drm", bufs=1, space="DRAM"))
    abp = ctx.enter_context(tc.tile_pool(name="apool", bufs=10))
    bp = ctx.enter_context(tc.tile_pool(name="bpool", bufs=10))
    cp = ctx.enter_context(tc.tile_pool(name="cpool", bufs=10))
    dp = ctx.enter_context(tc.tile_pool(name="dpool", bufs=25))
    scp = ctx.enter_context(tc.tile_pool(name="scr", bufs=3))

    dtab = drm.tile([ntokb + 3, 2 * EMB_DIM], dt.bfloat16)
    widx = cst.tile([P, nwin * wcols], dt.int16)
    gidx = cst.tile([P, N_LISTS * IDX_COLS], dt.int16)
    strips = []
    for si_ in range(N_STREAMS):
        strip = cst.tile([P, NCOLS], dt.float32, tag=f"strip{si_}", name=f"strip{si_}")
        strips.append(strip)

    nc.sync.dma_start(widx[:], widx_t[:])
    nc.sync.dma_start(gidx[:], gidx_t[:])

    qrr = [0]
    prev_g = [None]

    def nextq():
        q = qrr[0] % QUEUES
        qrr[0] += 1
        return q

    def chain(inst):
        # Pin scheduler emission order of SWDGE ops to program order so
        # Tile's DMA sem lanes (rr mod 8) stay aligned with the strict
        # queue round-robin (mod 4): ucode requires each sem lane to be
        # incremented from a single queue.
        if prev_g[0] is not None:
            add_dep_helper(inst.ins, prev_g[0].ins, False,
                           "swdge order chain")
        prev_g[0] = inst

    # Token-table build. Window i: one 1024-idx single-packet gather from its
    # HBM slice into a rotating staging tile (parallel across queues), then a
    # bulk DMA of the first 896 slots into the DRAM table. Pad tokens last.
    nc.sync.dma_start(
        dtab[ntokb:ntokb + 3, :].rearrange("(r p) e -> p r e", p=3), ptok_t[:])
    for i in range(nwin):
        lo = bases[i]
        hi = min(vocab, lo + WIN_ROWS)
        s = stg.tile([P, sranks, 2 * EMB_DIM], dt.bfloat16, tag="stg")
        g = nc.gpsimd.dma_gather(
            s[:], tok_t[lo:hi, :],
            widx[:, i * wcols:(i + 1) * wcols],
            WCALL, WCALL, 2 * EMB_DIM,
            single_packet=True,
            queue_num=nextq(),
        )
        chain(g)
        nc.sync.dma_start(
            dtab[padw * i:padw * (i + 1), :].rearrange("(r p) e -> p r e", p=P),
            s[:, :wranks, :])

    node_half = dtab[:, :EMB_DIM]
    ctx_half = dtab[:, EMB_DIM:]

    def gather(dst, half, cols, n):
        g = nc.gpsimd.dma_gather(
            dst[:], half, gidx[:, cols:cols + n // 16], n, n, EMB_DIM,
            elem_step=2 * EMB_DIM,
            single_packet=True,
            queue_num=nextq(),
        )
        chain(g)

    col0 = 0
    scol = 0
    for gi, n in enumerate(CALLS if not NO_MAIN else []):
        nr = n // P
        a = abp.tile([P, nr, EMB_DIM], dt.bfloat16, tag="a")
        gather(a, node_half, col0, n)
        b = bp.tile([P, nr, EMB_DIM], dt.bfloat16, tag="b")
        gather(b, ctx_half, col0, n)
        c = cp.tile([P, nr, EMB_DIM], dt.bfloat16, tag="c")
        gather(c, node_half, IDX_COLS + col0, n)
        ds = []
        for j in range(NEG_SIZE):
            d = dp.tile([P, nr, EMB_DIM], dt.bfloat16, tag="d")
            gather(d, ctx_half, (2 + j) * IDX_COLS + col0, n)
            ds.append(d)
        if not NO_COMPUTE:
            nc.vector.tensor_mul(c[:], c[:], b[:])
            nc.vector.tensor_reduce(strips[0][:, scol:scol + nr], c[:],
                                    axis=mybir.AxisListType.X,
                                    op=mybir.AluOpType.add)
            for j in range(NEG_SIZE):
                d = ds[j]
                nc.vector.tensor_mul(d[:], d[:], a[:])
                nc.vector.tensor_reduce(strips[1 + j][:, scol:scol + nr], d[:],
                                        axis=mybir.AxisListType.X,
                                        op=mybir.AluOpType.add)
        col0 += n // 16
        scol += nr

    accvs = []
    for s2 in range(N_STREAMS if not NO_COMPUTE and not NO_MAIN else 0):
        scale = -1.0 if s2 == 0 else 1.0   # pos stream: softplus(-score)
        e = scp.tile([P, NCOLS], dt.float32, tag="e")
        sp = scp.tile([P, NCOLS], dt.float32, tag="sp")
        av = cst.tile([P, 1], dt.float32, tag=f"av{s2}")
        nc.scalar.activation(e[:], strips[s2][:],
                             mybir.ActivationFunctionType.Exp, scale=scale)
        nc.scalar.activation(sp[:], e[:],
                             mybir.ActivationFunctionType.Ln,
                             bias=1.0, accum_out=av[:])
        accvs.append(av)
    osb = cst.tile([P, 1], dt.float32, tag="osb")
    nc.vector.memset(osb[:], 0.0)
    for av in accvs:
        nc.vector.tensor_add(osb[:], osb[:], av[:])
    nc.sync.dma_start(out_t[:], osb[:])


def _build_program(loop_k, vocab, padw, bases):
    nc = bacc.Bacc("TRN2", target_bir_lowering=False, debug=False,
                   num_swdge_queues=QUEUES)
    nwin = len(bases)
    tok_t = nc.dram_tensor("tok", [vocab, 2 * EMB_DIM], mybir.dt.bfloat16,
                           kind="ExternalInput")
    widx_t = nc.dram_tensor("widx", [P, nwin * WCALL // 16], mybir.dt.int16,
                            kind="ExternalInput")
    gidx_t = nc.dram_tensor("gidx", [P, N_LISTS * IDX_COLS], mybir.dt.int16,
                            kind="ExternalInput")
    ptok_t = nc.dram_tensor("ptok", [3, 2 * EMB_DIM], mybir.dt.bfloat16,
                            kind="ExternalInput")
    out_t = nc.dram_tensor("out", [P, 1], mybir.dt.float32, kind="ExternalOutput")
    with tile.TileContext(nc) as tc:
        nc.gpsimd.load_library(library_config.mlp)
        if loop_k is None:
            _body(tc, nc, tok_t, widx_t, gidx_t, ptok_t, out_t, vocab, padw, bases)
        else:
            tc.For_i_unrolled(0, loop_k, 1,
                              lambda iv: _body(tc, nc, tok_t, widx_t, gidx_t,
                                               ptok_t, out_t, vocab, padw, bases),
                              max_unroll=1)
    nc.compile()
    return nc


_CACHE = {}


def _get_program(loop_k, vocab, padw, bases):
    key = (loop_k, vocab, padw, bases, NO_COMPUTE, NO_MAIN, QUEUES)
    if key not in _CACHE:
        _CACHE[key] = _build_program(loop_k, vocab, padw, bases)
    return _CACHE[key]


def run_cores(inputs, loop_k=None):
    """Run the SPMD kernel; returns list of per-core [128,1] partial sums."""
    in_maps, padw, bases = _host_prepare(**inputs)
    vocab = int(np.asarray(inputs["node_embed"]).shape[0])
    nc = _get_program(loop_k, vocab, padw, bases)
    res = run_bass_kernel_spmd(nc, in_maps, core_ids=list(range(N_CORES)))
    return [res.results[i]["out"] for i in range(N_CORES)]


def kernel(batch_walk, neg_idx_dst, node_embed, context_embed):
    outs = run_cores(dict(batch_walk=batch_walk, neg_idx_dst=neg_idx_dst,
                          node_embed=node_embed, context_embed=context_embed))
    total = float(sum(float(o.sum()) for o in outs))
    return np.float32(total / (BATCH * NUM_PAIRS))
